# revision 18
# baseline (speedup 1.0000x reference)
import sys
sys.path.insert(0, "/opt/trn_rl_repo")
import math
import os
import numpy as np
import ml_dtypes

import concourse.bacc as bacc
import concourse.bass as bass
import concourse.mybir as mybir
import concourse.tile as tile
from concourse.bass_utils import run_bass_kernel_spmd
from concourse.masks import make_identity

bf16 = ml_dtypes.bfloat16
F32 = mybir.dt.float32
BF16 = mybir.dt.bfloat16
I16 = mybir.dt.int16

N = 50000
E = 800000
IN = 512
H1, D1 = 4, 64
HD1 = 256
H2, D2 = 1, 64
NCORES = 8
NSH = N // NCORES          # 6250 nodes per core
P = 128
NBLK = math.ceil(NSH / P)  # 49
LO = 32768                 # int16 gather index limit split
GCH = int(os.environ.get("K_GCH", "4"))
SP = bool(int(os.environ.get("K_SP", "1")))
EBUFS = int(os.environ.get("K_EBUFS", "2"))
EPS = int(os.environ.get("K_EPS", "2"))  # max tiles (x128 idxs) per dma_gather instruction
RW1 = 384                  # T1 row width in bf16: 256 feat | 4 el f32 | 4 er f32 | pad
RW2 = 128                  # T2 row width in bf16: 64 feat | el f32 | er f32 | pad


def _wrap16(idx):
    """[n] ints -> [128, n//16] int16 gather-index layout (16-partition wrap, x8 replicated)."""
    n = len(idx)
    assert n % 16 == 0
    a = np.asarray(idx, dtype=np.int16).reshape(n // 16, 16).T
    return np.tile(a, (8, 1))


def _prep_edges(src, dst):
    """Host-side edge sharding/ordering. Returns per-device index arrays + global schedule."""
    src = np.asarray(src).astype(np.int64)
    dst = np.asarray(dst).astype(np.int64)

    # per device, per block: lo/hi edge lists (sorted by dst within each)
    dev_lists = []  # [d][b] -> (lo_src, lo_dstoff, hi_src, hi_dstoff)
    for d in range(NCORES):
        m = (dst >= NSH * d) & (dst < NSH * (d + 1))
        s_d = src[m]
        t_d = dst[m] - NSH * d
        o = np.argsort(t_d, kind="stable")
        s_d, t_d = s_d[o], t_d[o]
        blk = t_d // P
        islo = s_d < LO
        blocks = []
        for b in range(NBLK):
            mb = blk == b
            sl, tl = s_d[mb & islo], t_d[mb & islo]
            sh, th = s_d[mb & ~islo], t_d[mb & ~islo]
            blocks.append((sl, tl - P * b, sh, th - P * b))
        dev_lists.append(blocks)

    nA = np.zeros(NBLK, dtype=np.int64)
    nB = np.zeros(NBLK, dtype=np.int64)
    for b in range(NBLK):
        for d in range(NCORES):
            sl, _, sh, _ = dev_lists[d][b]
            nA[b] = max(nA[b], (len(sl) + P - 1) // P)
            nB[b] = max(nB[b], (len(sh) + P - 1) // P)
        if nA[b] + nB[b] == 0:
            nA[b] = 1
    T = nA + nB
    NT = int(T.sum())

    idx_lo, idx_hi, idx_er, doff = [], [], [], []
    for d in range(NCORES):
        lo_cols, hi_cols, er_cols, do_cols = [], [], [], []
        for b in range(NBLK):
            sl, ol, sh, oh = dev_lists[d][b]
            npadA = nA[b] * P - len(sl)
            npadB = nB[b] * P - len(sh)
            lo_i = np.concatenate([sl, np.zeros(npadA, np.int64)])
            lo_o = np.concatenate([ol, np.full(npadA, -1.0)])
            hi_i = np.concatenate([sh - LO, np.zeros(npadB, np.int64)])
            hi_o = np.concatenate([oh, np.full(npadB, -1.0)])
            # er idx = local dst node id; dummies -> 0
            er_i = np.concatenate([
                ol + P * b, np.zeros(npadA, np.int64),
                oh + P * b, np.zeros(npadB, np.int64),
            ])
            if nA[b]:
                lo_cols.append(_wrap16(lo_i))
                er_cols.append(_wrap16(er_i[: nA[b] * P]))
            if nB[b]:
                hi_cols.append(_wrap16(hi_i))
                er_cols.append(_wrap16(er_i[nA[b] * P:]))
            do = np.concatenate([lo_o, hi_o]).astype(np.float32)
            do_cols.append(do.reshape(T[b], P).T)
        idx_lo.append(np.hstack(lo_cols).astype(np.int16) if lo_cols else np.zeros((128, 0), np.int16))
        idx_hi.append(np.hstack(hi_cols).astype(np.int16) if hi_cols else np.zeros((128, 0), np.int16))
        idx_er.append(np.hstack(er_cols).astype(np.int16))
        doff.append(np.hstack(do_cols).astype(np.float32))
    return nA, nB, NT, idx_lo, idx_hi, idx_er, doff


def _build(nA, nB, NT, CL, CH, CE, has_b1, has_b2):
    STAGE = int(os.environ.get("K_STAGE", "6"))
    NQ = int(os.environ.get("K_QUEUES", "4"))
    nc = bacc.Bacc("TRN2", target_bir_lowering=False, debug=False, num_devices=NCORES,
                   num_swdge_queues=NQ)
    qctr = [0]
    def nextq():
        q = qctr[0] % NQ
        qctr[0] += 1
        return q

    xT = nc.dram_tensor("xT", [IN, NSH], F32, kind="ExternalInput")
    w1 = nc.dram_tensor("w1", [IN, HD1], F32, kind="ExternalInput")
    w1t = nc.dram_tensor("w1t", [HD1, IN], F32, kind="ExternalInput")
    alar1 = nc.dram_tensor("alar1", [HD1, 8], F32, kind="ExternalInput")
    w2 = nc.dram_tensor("w2", [HD1, D2], F32, kind="ExternalInput")
    w2t = nc.dram_tensor("w2t", [D2, HD1], F32, kind="ExternalInput")
    alar2 = nc.dram_tensor("alar2", [D2, 2], F32, kind="ExternalInput")
    ilo = nc.dram_tensor("ilo", [128, max(CL, 1)], I16, kind="ExternalInput")
    ihi = nc.dram_tensor("ihi", [128, max(CH, 1)], I16, kind="ExternalInput")
    ier = nc.dram_tensor("ier", [128, CE], I16, kind="ExternalInput")
    idoff = nc.dram_tensor("idoff", [128, NT], F32, kind="ExternalInput")
    if has_b1:
        b1r = nc.dram_tensor("b1r", [128, HD1], F32, kind="ExternalInput")
    if has_b2:
        b2r = nc.dram_tensor("b2r", [128, D2], F32, kind="ExternalInput")
    out_t = nc.dram_tensor("out", [NSH, D2], F32, kind="ExternalOutput")

    iota_np = np.tile(np.arange(128, dtype=np.float32)[None, :], (128, 1))
    iota_d = nc.inline_tensor(iota_np, name="iota_c")

    ps_last = NSH - P * (NBLK - 1)  # rows in last block (106)

    with tile.TileContext(nc) as tc:
        with (
            tc.tile_pool(name="const", bufs=1) as cpool,
            tc.tile_pool(name="dram", bufs=1, space="DRAM") as dram,
        ):
            iota_t = cpool.tile([128, 128], F32)
            nc.sync.dma_start(out=iota_t[:], in_=iota_d[:, :])
            ident = cpool.tile([128, 128], BF16)
            make_identity(nc, ident[:])

            ilo_t = cpool.tile([128, max(CL, 1)], I16)
            ihi_t = cpool.tile([128, max(CH, 1)], I16)
            ier_t = cpool.tile([128, CE], I16)
            doff_t = cpool.tile([128, NT], F32)
            nc.sync.dma_start(out=ilo_t[:], in_=ilo[:, :])
            nc.sync.dma_start(out=ihi_t[:], in_=ihi[:, :])
            nc.sync.dma_start(out=ier_t[:], in_=ier[:, :])
            nc.sync.dma_start(out=doff_t[:], in_=idoff[:, :])
            if has_b1:
                b1_t = cpool.tile([128, HD1], F32)
                nc.sync.dma_start(out=b1_t[:], in_=b1r[:, :])
            if has_b2:
                b2_t = cpool.tile([128, D2], F32)
                nc.sync.dma_start(out=b2_t[:], in_=b2r[:, :])

            # persistent hT (transposed L1 output, input to dense L2)
            hT = []
            for k in range(2):
                hT_k = cpool.tile([128, NBLK * P], BF16, tag=f"hT{k}", name=f"hT{k}")
                hT.append(hT_k)

            T1_local = dram.tile([NSH, RW1], BF16)
            T1_full = dram.tile([N, RW1], BF16, addr_space="Shared")
            T2_local = dram.tile([NSH, RW2], BF16)
            T2_full = dram.tile([N, RW2], BF16, addr_space="Shared")

            # ---------------- phase 0+1: dense L1 (feat1/el1/er1 -> T1_local) --------
            with (
                tc.tile_pool(name="dsb", bufs=1) as dsb,
                tc.tile_pool(name="dps", bufs=2, space="PSUM") as dps,
                tc.tile_pool(name="combop", bufs=3) as combop,
            ):
                w1t_t = []
                for k in range(2):
                    w1t_k = dsb.tile([128, IN], BF16, tag=f"w1t{k}", name=f"w1t{k}")
                    w1t_t.append(w1t_k)
                alar1_t = []
                for k in range(2):
                    alar1_k = dsb.tile([128, 8], BF16, tag=f"alar1{k}", name=f"alar1{k}")
                    alar1_t.append(alar1_k)
                for k in range(2):
                    nc.gpsimd.dma_start(out=w1t_t[k][:], in_=w1t[128 * k:128 * (k + 1), :])
                    nc.gpsimd.dma_start(out=alar1_t[k][:], in_=alar1[128 * k:128 * (k + 1), :])
                rhsW1 = []
                for k in range(4):
                    rhsW1_k = dsb.tile([128, 264], BF16, tag=f"rhsW1{k}", name=f"rhsW1{k}")
                    rhsW1.append(rhsW1_k)
                for k in range(4):
                    nc.gpsimd.dma_start(out=rhsW1[k][:, 0:256], in_=w1[128 * k:128 * (k + 1), :])
                    psw = dps.tile([128, 8], F32, tag="psw")
                    for k2 in range(2):
                        nc.tensor.matmul(
                            out=psw[:], lhsT=w1t_t[k2][:, 128 * k:128 * (k + 1)],
                            rhs=alar1_t[k2][:], start=(k2 == 0), stop=(k2 == 1))
                    nc.vector.tensor_copy(rhsW1[k][:, 256:264], psw[:])

                xT_t = []
                for k in range(4):
                    xT_k = dsb.tile([128, NSH], BF16, tag=f"xT{k}", name=f"xT{k}")
                    xT_t.append(xT_k)
                for k in range(4):
                    nc.gpsimd.dma_start(out=xT_t[k][:], in_=xT[128 * k:128 * (k + 1), :])

                for nb in range(NBLK):
                    pb = P if nb < NBLK - 1 else ps_last
                    ps1 = dps.tile([128, 264], F32, tag="ps1")
                    for k in range(4):
                        nc.tensor.matmul(
                            out=ps1[:pb, :], lhsT=xT_t[k][:, P * nb:P * nb + pb],
                            rhs=rhsW1[k][:], start=(k == 0), stop=(k == 3))
                    combo = combop.tile([128, RW1], BF16, tag="combo1")
                    nc.gpsimd.memset(combo[:, 272:384], 0)
                    nc.vector.tensor_copy(combo[:pb, 0:256], ps1[:pb, 0:256])
                    nc.vector.tensor_copy(
                        combo[:pb, 256:272].bitcast(F32), ps1[:pb, 256:264])
                    nc.sync.dma_start(
                        out=T1_local[P * nb:P * nb + pb, :], in_=combo[:pb, :])

            # ---------------- phase 2: allgather T1 ----------------
            if STAGE >= 2:
              nc.gpsimd.collective_compute(
                "AllGather", mybir.AluOpType.bypass,
                replica_groups=[list(range(NCORES))],
                ins=[T1_local[:, :]], outs=[T1_full[:, :]])

            # ---------------- phase 3: L1 edge aggregation ----------------
            with (
                tc.tile_pool(name="esb", bufs=EBUFS) as esb,
                tc.tile_pool(name="esb3", bufs=4) as esb3,
                tc.tile_pool(name="eps", bufs=EPS, space="PSUM") as eps,
                tc.tile_pool(name="tps", bufs=2, space="PSUM") as tps,
            ):
                clo = chi = cer = ct = 0
                for b in range(NBLK if STAGE >= 3 else 0):
                    a, bb = int(nA[b]), int(nB[b])
                    t_b = a + bb
                    pb = P if b < NBLK - 1 else ps_last
                    bufA = bufB = None
                    if a:
                        bufA = esb.tile([128, a * RW1], BF16, tag="bufA")
                        if int(os.environ.get("K_NOF", "0")):
                            nc.gpsimd.memset(bufA[:, 0:128], 0)
                        for c0 in range(0, a if not int(os.environ.get("K_NOF", "0")) else 0, GCH):
                            cn = min(GCH, a - c0)
                            nc.gpsimd.dma_gather(
                                out_ap=bufA[:, RW1 * c0:RW1 * (c0 + cn)].rearrange("p (t e) -> p t e", e=RW1),
                                in_ap=T1_full[0:LO, :],
                                idxs_ap=ilo_t[:, clo + c0 * 8:clo + (c0 + cn) * 8],
                                num_idxs=cn * P, num_idxs_reg=cn * P, elem_size=RW1, queue_num=nextq(), single_packet=SP)
                    if bb:
                        bufB = esb.tile([128, bb * RW1], BF16, tag="bufB")
                        if int(os.environ.get("K_NOF", "0")):
                            nc.gpsimd.memset(bufB[:, 0:128], 0)
                        for c0 in range(0, bb if not int(os.environ.get("K_NOF", "0")) else 0, GCH):
                            cn = min(GCH, bb - c0)
                            nc.gpsimd.dma_gather(
                                out_ap=bufB[:, RW1 * c0:RW1 * (c0 + cn)].rearrange("p (t e) -> p t e", e=RW1),
                                in_ap=T1_full[LO:N, :],
                                idxs_ap=ihi_t[:, chi + c0 * 8:chi + (c0 + cn) * 8],
                                num_idxs=cn * P, num_idxs_reg=cn * P, elem_size=RW1, queue_num=nextq(), single_packet=SP)
                    bufE = esb.tile([128, t_b * 128], BF16, tag="bufE")
                    for c0 in range(0, t_b if not int(os.environ.get("K_NOER", "0")) else 0, GCH):
                        cn = min(GCH, t_b - c0)
                        nc.gpsimd.dma_gather(
                            out_ap=bufE[:, 128 * c0:128 * (c0 + cn)].rearrange("p (t e) -> p t e", e=128),
                            in_ap=T1_local[:, 256:384],
                            idxs_ap=ier_t[:, cer + c0 * 8:cer + (c0 + cn) * 8],
                            num_idxs=cn * P, num_idxs_reg=cn * P,
                            elem_size=128, elem_step=RW1, queue_num=nextq(), single_packet=SP)

                    # attention: z = el_src + er_dst ; ex = max(exp(z), exp(.2 z))
                    if int(os.environ.get("K_NOER", "0")):
                        nc.gpsimd.memset(bufE[:], 0)
                    z = esb.tile([128, t_b * 4], F32, tag="z")
                    zr = z[:].rearrange("p (t h) -> p t h", h=4)
                    er_r = bufE[:].bitcast(F32).rearrange("p (t c) -> p t c", c=64)
                    if a:
                        elA = bufA[:].bitcast(F32).rearrange("p (t c) -> p t c", c=192)
                        nc.vector.tensor_tensor(
                            out=zr[:, 0:a, :], in0=elA[:, :, 128:132],
                            in1=er_r[:, 0:a, 4:8], op=mybir.AluOpType.add)
                    if bb:
                        elB = bufB[:].bitcast(F32).rearrange("p (t c) -> p t c", c=192)
                        nc.vector.tensor_tensor(
                            out=zr[:, a:t_b, :], in0=elB[:, :, 128:132],
                            in1=er_r[:, a:t_b, 4:8], op=mybir.AluOpType.add)
                    e1 = esb.tile([128, t_b * 4], F32, tag="e1")
                    ex32 = esb.tile([128, t_b * 4], F32, tag="ex32")
                    nc.scalar.activation(out=e1[:], in_=z[:], func=mybir.ActivationFunctionType.Exp)
                    nc.scalar.activation(out=ex32[:], in_=z[:], func=mybir.ActivationFunctionType.Exp, scale=0.2)
                    nc.vector.tensor_tensor(out=ex32[:], in0=e1[:], in1=ex32[:], op=mybir.AluOpType.max)
                    rhs = esb.tile([128, t_b * 260], BF16, tag="rhs")
                    nc.vector.tensor_copy(
                        rhs[:].rearrange("p (t c) -> p t c", c=260)[:, :, 256:260],
                        ex32[:].rearrange("p (t h) -> p t h", h=4))

                    ps_o = eps.tile([128, 260], F32, tag="ps_o")
                    for t in range(t_b):
                        buf, blk = (bufA, t) if t < a else (bufB, t - a)
                        S_t = esb3.tile([128, 128], BF16, tag="S")
                        nc.vector.tensor_scalar(
                            out=S_t[:], in0=iota_t[:], scalar1=doff_t[:, ct + t:ct + t + 1],
                            scalar2=None, op0=mybir.AluOpType.is_equal)
                        for h in (0, 1):
                            nc.vector.tensor_scalar(
                                out=rhs[:, 260 * t + 64 * h:260 * t + 64 * h + 64],
                                in0=buf[:, RW1 * blk + 64 * h:RW1 * blk + 64 * h + 64],
                                scalar1=ex32[:, 4 * t + h:4 * t + h + 1],
                                scalar2=None, op0=mybir.AluOpType.mult)
                        for h in (2, 3):
                            nc.scalar.activation(
                                out=rhs[:, 260 * t + 64 * h:260 * t + 64 * h + 64],
                                in_=buf[:, RW1 * blk + 64 * h:RW1 * blk + 64 * h + 64],
                                func=mybir.ActivationFunctionType.Copy,
                                scale=ex32[:, 4 * t + h:4 * t + h + 1])
                        nc.tensor.matmul(
                            out=ps_o[:], lhsT=S_t[:], rhs=rhs[:, 260 * t:260 * t + 260],
                            start=(t == 0), stop=(t == t_b - 1))

                    # normalize + elu -> h block (bf16) -> transpose into hT
                    splus = esb.tile([128, 4], F32, tag="splus")
                    nc.vector.tensor_scalar(
                        out=splus[:], in0=ps_o[:, 256:260], scalar1=1e-30,
                        scalar2=None, op0=mybir.AluOpType.add)
                    r = esb.tile([128, 4], F32, tag="r")
                    nc.vector.reciprocal(r[:], splus[:])
                    xn = esb.tile([128, 256], F32, tag="xn")
                    r_b = bass.AP(r[:].tensor, r[:].offset, [r[:].ap[0], [1, 4], [0, 64]])
                    nc.vector.tensor_tensor(
                        out=xn[:].rearrange("p (h d) -> p h d", h=4),
                        in0=ps_o[:, 0:256].rearrange("p (h d) -> p h d", h=4),
                        in1=r_b, op=mybir.AluOpType.mult)
                    if has_b1:
                        nc.vector.tensor_tensor(out=xn[:], in0=xn[:], in1=b1_t[:], op=mybir.AluOpType.add)
                    # elu(x) = exp(min(x,0)) + (max(x,0) - 1)
                    t1 = esb.tile([128, 256], F32, tag="t1")
                    nc.vector.tensor_scalar(
                        out=t1[:], in0=xn[:], scalar1=0.0, scalar2=None, op0=mybir.AluOpType.min)
                    u = esb.tile([128, 256], F32, tag="u")
                    nc.scalar.activation(out=u[:], in_=t1[:], func=mybir.ActivationFunctionType.Exp)
                    v = esb.tile([128, 256], F32, tag="v")
                    nc.vector.tensor_scalar(
                        out=v[:], in0=xn[:], scalar1=0.0, scalar2=-1.0,
                        op0=mybir.AluOpType.max, op1=mybir.AluOpType.add)
                    hb = esb.tile([128, 256], BF16, tag="hb")
                    nc.vector.tensor_tensor(out=hb[:], in0=u[:], in1=v[:], op=mybir.AluOpType.add)
                    for k2 in range(2):
                        pst = tps.tile([128, 128], BF16, tag="pst")
                        nc.tensor.transpose(out=pst[:], in_=hb[:, 128 * k2:128 * (k2 + 1)], identity=ident[:])
                        nc.vector.tensor_copy(hT[k2][:, P * b:P * (b + 1)], pst[:])

                    clo += a * 8
                    chi += bb * 8
                    cer += t_b * 8
                    ct += t_b

            # ---------------- phase 4: dense L2 ----------------
            with (
                tc.tile_pool(name="d2sb", bufs=1) as d2sb,
                tc.tile_pool(name="d2ps", bufs=2, space="PSUM") as d2ps,
                tc.tile_pool(name="combop2", bufs=3) as combop2,
            ):
                w2t_t = d2sb.tile([128, HD1], BF16, tag="w2t")
                alar2_t = d2sb.tile([128, 2], BF16, tag="alar2")
                nc.gpsimd.dma_start(out=w2t_t[:64, :], in_=w2t[:, :])
                nc.gpsimd.dma_start(out=alar2_t[:64, :], in_=alar2[:, :])
                rhsW2 = []
                for k in range(2):
                    rhsW2_k = d2sb.tile([128, 66], BF16, tag=f"rhsW2{k}", name=f"rhsW2{k}")
                    rhsW2.append(rhsW2_k)
                for k in range(2):
                    nc.gpsimd.dma_start(out=rhsW2[k][:, 0:64], in_=w2[128 * k:128 * (k + 1), :])
                    psw2 = d2ps.tile([128, 2], F32, tag="psw2")
                    nc.tensor.matmul(
                        out=psw2[:], lhsT=w2t_t[:64, 128 * k:128 * (k + 1)],
                        rhs=alar2_t[:64, :], start=True, stop=True)
                    nc.vector.tensor_copy(rhsW2[k][:, 64:66], psw2[:])

                for nb in range(NBLK if STAGE >= 4 else 0):
                    pb = P if nb < NBLK - 1 else ps_last
                    ps2 = d2ps.tile([128, 66], F32, tag="ps2")
                    for k in range(2):
                        nc.tensor.matmul(
                            out=ps2[:pb, :], lhsT=hT[k][:, P * nb:P * nb + pb],
                            rhs=rhsW2[k][:], start=(k == 0), stop=(k == 1))
                    combo2 = combop2.tile([128, RW2], BF16, tag="combo2")
                    nc.gpsimd.memset(combo2[:, 68:128], 0)
                    nc.vector.tensor_copy(combo2[:pb, 0:64], ps2[:pb, 0:64])
                    nc.vector.tensor_copy(combo2[:pb, 64:68].bitcast(F32), ps2[:pb, 64:66])
                    nc.sync.dma_start(out=T2_local[P * nb:P * nb + pb, :], in_=combo2[:pb, :])

            # ---------------- phase 5: allgather T2 ----------------
            if STAGE >= 5:
              nc.gpsimd.collective_compute(
                "AllGather", mybir.AluOpType.bypass,
                replica_groups=[list(range(NCORES))],
                ins=[T2_local[:, :]], outs=[T2_full[:, :]])

            # ---------------- phase 6: L2 edge aggregation ----------------
            with (
                tc.tile_pool(name="e2sb", bufs=EBUFS) as e2sb,
                tc.tile_pool(name="e2sb3", bufs=4) as e2sb3,
                tc.tile_pool(name="e2ps", bufs=EPS, space="PSUM") as e2ps,
            ):
                clo = chi = cer = ct = 0
                for b in range(NBLK if STAGE >= 6 else 0):
                    a, bb = int(nA[b]), int(nB[b])
                    t_b = a + bb
                    pb = P if b < NBLK - 1 else ps_last
                    bufA = bufB = None
                    if a:
                        bufA = e2sb.tile([128, a * RW2], BF16, tag="bufA2")
                        for c0 in range(0, a, GCH):
                            cn = min(GCH, a - c0)
                            nc.gpsimd.dma_gather(
                                out_ap=bufA[:, RW2 * c0:RW2 * (c0 + cn)].rearrange("p (t e) -> p t e", e=RW2),
                                in_ap=T2_full[0:LO, :],
                                idxs_ap=ilo_t[:, clo + c0 * 8:clo + (c0 + cn) * 8],
                                num_idxs=cn * P, num_idxs_reg=cn * P, elem_size=RW2, queue_num=nextq(), single_packet=SP)
                    if bb:
                        bufB = e2sb.tile([128, bb * RW2], BF16, tag="bufB2")
                        for c0 in range(0, bb, GCH):
                            cn = min(GCH, bb - c0)
                            nc.gpsimd.dma_gather(
                                out_ap=bufB[:, RW2 * c0:RW2 * (c0 + cn)].rearrange("p (t e) -> p t e", e=RW2),
                                in_ap=T2_full[LO:N, :],
                                idxs_ap=ihi_t[:, chi + c0 * 8:chi + (c0 + cn) * 8],
                                num_idxs=cn * P, num_idxs_reg=cn * P, elem_size=RW2, queue_num=nextq(), single_packet=SP)
                    bufE = e2sb.tile([128, t_b * RW2], BF16, tag="bufE2")
                    for c0 in range(0, t_b if not int(os.environ.get("K_NOER", "0")) else 0, GCH):
                        cn = min(GCH, t_b - c0)
                        nc.gpsimd.dma_gather(
                            out_ap=bufE[:, RW2 * c0:RW2 * (c0 + cn)].rearrange("p (t e) -> p t e", e=RW2),
                            in_ap=T2_local[:, :],
                            idxs_ap=ier_t[:, cer + c0 * 8:cer + (c0 + cn) * 8],
                            num_idxs=cn * P, num_idxs_reg=cn * P, elem_size=RW2, queue_num=nextq(), single_packet=SP)

                    if int(os.environ.get("K_NOER", "0")):
                        nc.gpsimd.memset(bufE[:], 0)
                    z = e2sb.tile([128, t_b], F32, tag="z2")
                    zr = z[:].rearrange("p (t h) -> p t h", h=1)
                    er_r = bufE[:].bitcast(F32).rearrange("p (t c) -> p t c", c=64)
                    if a:
                        elA = bufA[:].bitcast(F32).rearrange("p (t c) -> p t c", c=64)
                        nc.vector.tensor_tensor(
                            out=zr[:, 0:a, :], in0=elA[:, :, 32:33],
                            in1=er_r[:, 0:a, 33:34], op=mybir.AluOpType.add)
                    if bb:
                        elB = bufB[:].bitcast(F32).rearrange("p (t c) -> p t c", c=64)
                        nc.vector.tensor_tensor(
                            out=zr[:, a:t_b, :], in0=elB[:, :, 32:33],
                            in1=er_r[:, a:t_b, 33:34], op=mybir.AluOpType.add)
                    e1 = e2sb.tile([128, t_b], F32, tag="e12")
                    ex32 = e2sb.tile([128, t_b], F32, tag="ex322")
                    nc.scalar.activation(out=e1[:], in_=z[:], func=mybir.ActivationFunctionType.Exp)
                    nc.scalar.activation(out=ex32[:], in_=z[:], func=mybir.ActivationFunctionType.Exp, scale=0.2)
                    nc.vector.tensor_tensor(out=ex32[:], in0=e1[:], in1=ex32[:], op=mybir.AluOpType.max)
                    rhs = e2sb.tile([128, t_b * 65], BF16, tag="rhs2")
                    nc.vector.tensor_copy(
                        rhs[:].rearrange("p (t c) -> p t c", c=65)[:, :, 64:65],
                        ex32[:].rearrange("p (t h) -> p t h", h=1))

                    ps_o = e2ps.tile([128, 65], F32, tag="ps_o2")
                    for t in range(t_b):
                        buf, blk = (bufA, t) if t < a else (bufB, t - a)
                        S_t = e2sb3.tile([128, 128], BF16, tag="S2")
                        nc.vector.tensor_scalar(
                            out=S_t[:], in0=iota_t[:], scalar1=doff_t[:, ct + t:ct + t + 1],
                            scalar2=None, op0=mybir.AluOpType.is_equal)
                        nc.vector.tensor_scalar(
                            out=rhs[:, 65 * t:65 * t + 64],
                            in0=buf[:, RW2 * blk:RW2 * blk + 64],
                            scalar1=ex32[:, t:t + 1], scalar2=None,
                            op0=mybir.AluOpType.mult)
                        nc.tensor.matmul(
                            out=ps_o[:], lhsT=S_t[:], rhs=rhs[:, 65 * t:65 * t + 65],
                            start=(t == 0), stop=(t == t_b - 1))

                    splus = e2sb.tile([128, 1], F32, tag="splus2")
                    nc.vector.tensor_scalar(
                        out=splus[:], in0=ps_o[:, 64:65], scalar1=1e-30,
                        scalar2=None, op0=mybir.AluOpType.add)
                    r = e2sb.tile([128, 1], F32, tag="r2")
                    nc.vector.reciprocal(r[:], splus[:])
                    outf = e2sb.tile([128, 64], F32, tag="outf")
                    nc.vector.tensor_scalar(
                        out=outf[:], in0=ps_o[:, 0:64], scalar1=r[:, 0:1],
                        scalar2=None, op0=mybir.AluOpType.mult)
                    if has_b2:
                        nc.vector.tensor_tensor(out=outf[:], in0=outf[:], in1=b2_t[:], op=mybir.AluOpType.add)
                    nc.sync.dma_start(out=out_t[P * b:P * b + pb, :], in_=outf[:pb, :])

                    clo += a * 8
                    chi += bb * 8
                    cer += t_b * 8
                    ct += t_b

            if STAGE < 6:
                zer = cpool.tile([128, D2], F32, name="zer")
                nc.gpsimd.memset(zer[:], 0)
                for b in range(NBLK):
                    pb = P if b < NBLK - 1 else ps_last
                    nc.sync.dma_start(out=out_t[P * b:P * b + pb, :], in_=zer[:pb, :])

    nc.compile()
    return nc


def kernel(x, src, dst, W1, al1, ar1, b1, W2, al2, ar2, b2):
    x = np.asarray(x, dtype=np.float32)
    W1 = np.asarray(W1, dtype=np.float32)
    al1 = np.asarray(al1, dtype=np.float32)
    ar1 = np.asarray(ar1, dtype=np.float32)
    b1 = np.asarray(b1, dtype=np.float32)
    W2 = np.asarray(W2, dtype=np.float32)
    al2 = np.asarray(al2, dtype=np.float32)
    ar2 = np.asarray(ar2, dtype=np.float32)
    b2 = np.asarray(b2, dtype=np.float32)

    nA, nB, NT, idx_lo, idx_hi, idx_er, doff = _prep_edges(src, dst)
    CL, CH, CE = idx_lo[0].shape[1], idx_hi[0].shape[1], idx_er[0].shape[1]
    has_b1 = bool(np.any(b1))
    has_b2 = bool(np.any(b2))

    nc = _build(nA, nB, NT, CL, CH, CE, has_b1, has_b2)

    # replicated params
    alar1_np = np.zeros((HD1, 8), np.float32)
    for h in range(H1):
        alar1_np[64 * h:64 * (h + 1), h] = al1[h]
        alar1_np[64 * h:64 * (h + 1), 4 + h] = ar1[h]
    alar2_np = np.zeros((D2, 2), np.float32)
    alar2_np[:, 0] = al2[0]
    alar2_np[:, 1] = ar2[0]
    w1t_np = np.ascontiguousarray(W1.T)
    w2t_np = np.ascontiguousarray(W2.T)
    xt_np = np.ascontiguousarray(x.T)

    in_maps = []
    for d in range(NCORES):
        m = {
            "xT": np.ascontiguousarray(xt_np[:, NSH * d:NSH * (d + 1)]),
            "w1": W1, "w1t": w1t_np, "alar1": alar1_np,
            "w2": W2, "w2t": w2t_np, "alar2": alar2_np,
            "ilo": np.ascontiguousarray(idx_lo[d]) if CL else np.zeros((128, 1), np.int16),
            "ihi": np.ascontiguousarray(idx_hi[d]) if CH else np.zeros((128, 1), np.int16),
            "ier": np.ascontiguousarray(idx_er[d]),
            "idoff": np.ascontiguousarray(doff[d]),
        }
        if has_b1:
            m["b1r"] = np.tile(b1.reshape(1, HD1), (128, 1)).astype(np.float32)
        if has_b2:
            m["b2r"] = np.tile(b2.reshape(1, D2), (128, 1)).astype(np.float32)
        in_maps.append(m)

    trace = bool(int(os.environ.get("K_TRACE", "0")))
    res = run_bass_kernel_spmd(
        nc, in_maps, core_ids=list(range(NCORES)), trace=trace,
        trace_cores=list(range(NCORES)) if trace else None, stitch_traces=trace)
    if trace:
        print("exec_time_ns:", res.exec_time_ns,
              "mean:", res.mean_exec_time_ns,
              "per_core_scope:", res.per_core_scope_times)
        print("trace:", res.instructions_and_trace[1] if res.instructions_and_trace else None)
        print("profile_json:", res.profile_json)
    out = np.concatenate([res.results[d]["out"] for d in range(NCORES)], axis=0)
    return out
